# revision 18
# baseline (speedup 1.0000x reference)
"""Trainium2 Bass kernel for CRF logZ (nn_CRFModel).

Math: probability-space recurrence with a 1/64 rescale folded into the
transitions (expAs = exp(WA - log64), masked); state stays ~[1e-5, 1e-1]
so no per-step normalization is needed.  logZ = log(z) + 129*log64.

Structure:

1. Rank-64 lexicon: emis = ThetaB @ E.T has rank <= 64.  Host computes
   ThetaB.T = Q @ R (QR) and Ep = E @ Q [V, 64] fp16 once; the device
   applies R on-chip (stationary blockdiag(R, R)).

2. Fwd/bwd meet-in-the-middle: z = beta_64^T p_64 with p running
   forward from BOS and gamma backward from the EOS column, both packed
   into one [128, 32] tile (fwd tags on partitions 0:64, bwd on 64:128)
   with a block-diagonal stationary [expAs, 0; 0, expAs^T].  64 rounds
   of one matmul + one DVE multiply replace 128 rounds of two each.

3. Hybrid emission delivery: the first 32 rounds + the backward-init
   column are staged by the host as dense Ep-row tiles (the host
   computes the gather indices from `words` anyway) and DMA'd over the
   fast hardware queue, so the recurrence starts ~11us in, while the
   Q7 SWDGE ucode (~10us library load) warms up.  Rounds 32-63 use
   device dma_gathers of 256-byte pair-rows Ep2[w//2] = [Ep[2r];
   Ep[2r+1]] (idx fits int16 since V/2 < 32768) in graded groups
   (8/16/8 rounds) spread over 4 SWDGE queues so each group lands
   before the recurrence reaches it.  Parity select (which half of a
   pair-row a word needs) runs post-exp: two activations produce
   even/odd candidates, one copy_predicated keeps the right ones for
   fwd+bwd at once.  Order-only add_dep anchors keep prep instructions
   from being scheduled ahead of earlier rounds (in-order engines).
"""

import sys

for _p in ("/opt/trn_rl_repo", "/root/.axon_site/_ro/trn_rl_repo"):
    if _p not in sys.path:
        sys.path.insert(0, _p)

import math

import numpy as np

import concourse.bass as bass
import concourse.mybir as mybir
import concourse.tile as tile
from concourse import bacc
from concourse.bass_utils import run_bass_kernel_spmd
from concourse.tile import add_dep_helper

K = 64
V = 50257
V2 = 50258              # padded even
D = 512
BT = 256
T = 128
BOS = 62
EOS = 63
N_CORES = 8
B = BT // N_CORES       # 32 sentences per core
NR = 64                 # rounds (fwd/bwd meet in the middle)
DR = 32                 # dense (host-staged) rounds
LOG64 = math.log(64.0)
NEG = -1e30

# gathered groups: (start_round, n_rounds, prep_anchor_round)
GGROUPS = [(32, 8, 22), (40, 16, 30), (56, 8, 44)]
N_IDX = sum(2 * n * B for _, n, _ in GGROUPS)   # 2048
S_IDX = N_IDX // 16                             # 128

F32 = mybir.dt.float32
F16 = mybir.dt.float16
I16 = mybir.dt.int16
U8 = mybir.dt.uint8

_CACHE = {}


def _build():
    nc = bacc.Bacc("TRN2", target_bir_lowering=False, debug=False,
                   num_devices=N_CORES, num_swdge_queues=4)

    idx_d = nc.dram_tensor("idx", [128, S_IDX], I16, kind="ExternalInput").ap()
    msk_d = nc.dram_tensor("msk", [128, N_IDX // 2], U8,
                           kind="ExternalInput").ap()
    g0i_d = nc.dram_tensor("g0i", [128, DR * B + B], F16,
                           kind="ExternalInput").ap()
    bd_d = nc.dram_tensor("bd", [128, 128], F16, kind="ExternalInput").ap()
    wrr_d = nc.dram_tensor("wrr", [128, 128], F16, kind="ExternalInput").ap()
    p0_d = nc.dram_tensor("p0", [K, B], F16, kind="ExternalInput").ap()
    lnc_d = nc.dram_tensor("lnc", [128, 1], F32, kind="ExternalInput").ap()
    ep2_d = nc.dram_tensor("ep2", [V2 // 2, 128], F16,
                           kind="ExternalInput").ap()
    out_d = nc.dram_tensor("out", [1, B], F32, kind="ExternalOutput").ap()

    with tile.TileContext(nc) as tc:
        with (
            tc.tile_pool(name="const", bufs=1) as cpool,
            tc.tile_pool(name="st", bufs=3) as spool,
            tc.tile_pool(name="psum_em", bufs=2, space="PSUM") as ps_em,
            tc.tile_pool(name="psum_q", bufs=3, space="PSUM") as ps_q,
        ):
            HB = DR * B // 2
            # ---- host-staged dense tiles, earliest-needed first ----------
            g0i = cpool.tile([128, DR * B + B], F16, tag="g0i")
            nc.scalar.dma_start(g0i[:, DR * B:DR * B + B],
                                g0i_d[:, DR * B:DR * B + B])
            QB = DR * B // 4
            nc.scalar.dma_start(g0i[:, 0:QB], g0i_d[:, 0:QB])
            wrr = cpool.tile([128, 128], F16, tag="wrr")
            nc.scalar.dma_start(wrr[:], wrr_d[:])
            lnc = cpool.tile([128, 1], F32, tag="lnc")
            nc.scalar.dma_start(lnc[:], lnc_d[:])
            p0 = cpool.tile([K, B], F16, tag="p0s")
            nc.scalar.dma_start(p0[:], p0_d[:])
            nc.scalar.dma_start(g0i[:, QB:2 * QB], g0i_d[:, QB:2 * QB])
            bd = cpool.tile([128, 128], F16, tag="bd")
            nc.scalar.dma_start(bd[:], bd_d[:])
            nc.scalar.dma_start(g0i[:, 2 * QB:3 * QB], g0i_d[:, 2 * QB:3 * QB])
            idx = cpool.tile([128, S_IDX], I16, tag="idx")
            nc.scalar.dma_start(idx[0:32, :], idx_d[0:32, :])
            nc.vector.tensor_copy(idx[32:64, :], idx[0:32, :])
            nc.vector.tensor_copy(idx[64:128, :], idx[0:64, :])
            nc.scalar.dma_start(g0i[:, 3 * QB:4 * QB], g0i_d[:, 3 * QB:4 * QB])
            msk = cpool.tile([128, N_IDX // 2], U8, tag="msk")
            nc.scalar.dma_start(msk[:], msk_d[:])
            ones = cpool.tile([K, 1], F16, tag="ones")
            nc.vector.memset(ones[:], 1.0)

            # ---- device gathers (graded groups over the 4 SWDGE queues),
            # pinned early so the ~10us Q7 ucode load starts immediately ---
            hp = tc.high_priority()
            hp.__enter__()
            r512 = nc.gpsimd.to_reg(512)
            idx0 = cpool.tile([128, 8], I16, tag="idx0")
            nc.vector.memset(idx0[:], 0)
            warm = cpool.tile([128, 128], F16, tag="warm")
            nc.gpsimd.dma_gather(
                warm[:].rearrange("p (c w) -> p c w", c=1),
                ep2_d[:], idx0[:], 128, nc.gpsimd.to_reg(128),
                128, transpose=True, queue_num=0)
            gts = []
            qn = 0
            ioff = 0
            for (g0r, n, _) in GGROUPS:
                gt = cpool.tile([128, 2 * n * B], F16, tag=f"gt{g0r}")
                half = n * B
                if half <= 256:
                    # one 512-idx gather covers fwd+bwd
                    nc.gpsimd.dma_gather(
                        gt[:].rearrange("p (c w) -> p c w", c=1),
                        ep2_d[:], idx[:, ioff:ioff + 32], 512, r512,
                        128, transpose=True, queue_num=qn % 4)
                    qn += 1
                    ioff += 32
                else:
                    nc.gpsimd.dma_gather(
                        gt[:, 0:half].rearrange("p (c w) -> p c w", c=1),
                        ep2_d[:], idx[:, ioff:ioff + 32], 512, r512,
                        128, transpose=True, queue_num=qn % 4)
                    nc.gpsimd.dma_gather(
                        gt[:, half:2 * half].rearrange("p (c w) -> p c w",
                                                       c=1),
                        ep2_d[:], idx[:, ioff + 32:ioff + 64], 512, r512,
                        128, transpose=True, queue_num=(qn + 1) % 4)
                    qn += 2
                    ioff += 64
                gts.append(gt)
            hp.__exit__(None, None, None)

            # ---- init: S0 = [p0 ; gamma_127] ----------------------------
            # gamma_127 = exp(emis(word[:,127]) + ln expAs[:, EOS])
            S = cpool.tile([128, B], F16, tag="S0")
            nc.vector.tensor_copy(S[0:K, :], p0[:])
            em_i = ps_q.tile([128, B], F32, tag="q")
            nc.tensor.matmul(em_i[:], lhsT=wrr[:],
                             rhs=g0i[:, DR * B:DR * B + B],
                             start=True, stop=True)
            nc.scalar.activation(S[K:128, :], em_i[K:128, :],
                                 mybir.ActivationFunctionType.Exp,
                                 bias=lnc[K:128, :], scale=1.0)

            # ---- emissions ----------------------------------------------
            expe_all = cpool.tile([128, NR * B], F16, tag="expe")
            cand_all = cpool.tile([128, N_IDX // 2], F16, tag="cand")

            # dense rounds: one GEMM + one exp per 256 columns
            for h in range(4):
                emd = ps_em.tile([128, QB], F32, tag="em")
                nc.tensor.matmul(emd[:], lhsT=wrr[:],
                                 rhs=g0i[:, h * QB:(h + 1) * QB],
                                 start=True, stop=True)
                nc.scalar.activation(expe_all[:, h * QB:(h + 1) * QB],
                                     emd[:],
                                     mybir.ActivationFunctionType.Exp)
            nc.tensor.ldweights(bd[:])

            coff = [0]

            def prep(gi, anchor):
                g0r, n, _ = GGROUPS[gi]
                gt = gts[gi]
                half = n * B
                expe = expe_all[:, g0r * B:(g0r + n) * B]
                cand = cand_all[:, coff[0]:coff[0] + half]
                msl = msk[:, coff[0]:coff[0] + half]
                coff[0] += half
                emf = ps_em.tile([128, half], F32, tag="em")
                mf = nc.tensor.matmul(emf[:], lhsT=wrr[:], rhs=gt[:, 0:half],
                                      start=True, stop=True)
                add_dep_helper(mf.ins, anchor.ins,
                               reason="keep prep gemm out of early rounds")
                nc.scalar.activation(expe[0:K], emf[0:K, :],
                                     mybir.ActivationFunctionType.Exp)
                nc.scalar.activation(cand[0:K], emf[K:128, :],
                                     mybir.ActivationFunctionType.Exp)
                emb = ps_em.tile([128, half], F32, tag="em")
                mb = nc.tensor.matmul(emb[:], lhsT=wrr[:],
                                      rhs=gt[:, half:2 * half],
                                      start=True, stop=True)
                add_dep_helper(mb.ins, anchor.ins,
                               reason="keep prep gemm out of early rounds")
                nc.scalar.activation(expe[K:128], emb[0:K, :],
                                     mybir.ActivationFunctionType.Exp)
                nc.scalar.activation(cand[K:128], emb[K:128, :],
                                     mybir.ActivationFunctionType.Exp)
                cp = nc.vector.copy_predicated(expe[:], msl[:], cand[:])
                add_dep_helper(cp.ins, anchor.ins,
                               reason="keep select out of early rounds")
                # restore the recurrence stationary after wrr clobbered it
                nc.tensor.ldweights(bd[:])

            anchors = {a: gi for gi, (_, _, a) in enumerate(GGROUPS)}

            # ---- 64 rounds -----------------------------------------------
            q_last = None
            for r in range(NR):
                q = ps_q.tile([128, B], F32, tag="q")
                mm = nc.tensor.matmul(q[:], lhsT=bd[:], rhs=S[:],
                                      start=True, stop=True)
                mm.ins.ldweights = False
                S = spool.tile([128, B], F16, tag="S")
                mul = nc.vector.tensor_mul(S[:], q[:],
                                           expe_all[:, r * B:(r + 1) * B])
                if r in anchors:
                    prep(anchors[r], mul)
                q_last = q

            # ---- tail ----------------------------------------------------
            # S = [p_64 ; junk], q_last = [q63 ; beta_64]
            t = cpool.tile([K, B], F16, tag="t")
            nc.vector.tensor_mul(t[:], S[0:K, :], q_last[K:128, :])
            z = ps_q.tile([1, B], F32, tag="q")
            nc.tensor.matmul(z[:], lhsT=ones[:], rhs=t[:], start=True,
                             stop=True)
            lnz = cpool.tile([1, B], F32, tag="lnz")
            nc.scalar.activation(lnz[:], z[:], mybir.ActivationFunctionType.Ln)
            res = cpool.tile([1, B], F32, tag="res")
            nc.vector.tensor_scalar_add(res[:], lnz[:], float((T + 1) * LOG64))
            nc.scalar.dma_start(out_d[:], res[:])

    nc.compile()
    return nc


def _get_nc():
    if "nc" not in _CACHE:
        _CACHE["nc"] = _build()
    return _CACHE["nc"]


def _wrap16(w):
    """idx j -> partition j%16, slot j//16; replicated to all 8 Q7 cores."""
    a = np.asarray(w, np.int16).reshape(-1, 16).T  # [16, S]
    return np.tile(a, (8, 1))                      # [128, S]


def _host_prep(WA, ThetaB, E):
    WA = np.asarray(WA, np.float32)
    ThetaB = np.asarray(ThetaB, np.float32)
    E = np.asarray(E, np.float32)

    Q, R = np.linalg.qr(ThetaB.T)                 # ThetaB.T = Q @ R
    Ep = (E @ Q).astype(np.float16)               # [V, 64]
    Ep = np.concatenate([Ep, np.zeros((V2 - V, K), np.float16)], axis=0)
    Ep2 = np.ascontiguousarray(Ep.reshape(V2 // 2, 128))

    expAs = np.exp(WA - LOG64).astype(np.float32)
    expAs[:, BOS] = 0.0
    expAs[EOS, :] = 0.0
    expAs16 = expAs.astype(np.float16)

    bd = np.zeros((128, 128), np.float16)
    bd[0:K, 0:K] = expAs16
    bd[K:128, K:128] = expAs16.T

    wrr = np.zeros((128, 128), np.float16)
    wrr[0:K, 0:K] = R.astype(np.float16)
    wrr[K:128, K:128] = R.astype(np.float16)

    p0 = np.zeros((K, B), np.float16)
    p0[BOS, :] = 1.0

    lnc = np.zeros((128, 1), np.float32)
    col = (WA[:, EOS] - LOG64).astype(np.float32)
    col[EOS] = NEG
    lnc[0:K, 0] = col
    lnc[K:128, 0] = col
    return Ep, Ep2, bd, wrr, p0, lnc


def _make_in_maps(words, WA, ThetaB, E):
    words = np.asarray(words)
    Ep, Ep2, bd, wrr, p0, lnc = _host_prep(WA, ThetaB, E)

    in_maps = []
    for c in range(N_CORES):
        wb = words[c * B:(c + 1) * B].astype(np.int64)  # [32, 128]
        fw = wb[:, 0:NR].T                              # [64, 32] fwd words
        bwcols = [126 - r for r in range(NR - 1)] + [63]
        bw = wb[:, bwcols].T                            # [64, 32] bwd words

        # dense rounds 0..DR-1 + init column
        g0i = np.zeros((128, DR * B + B), np.float16)
        g0i[0:K, 0:DR * B] = Ep[fw[0:DR].reshape(-1)].T
        g0i[K:128, 0:DR * B] = Ep[bw[0:DR].reshape(-1)].T
        g0i[K:128, DR * B:DR * B + B] = Ep[wb[:, 127]].T

        # gathered groups
        idx_parts = []
        m = np.zeros((128, N_IDX // 2), np.uint8)
        coff = 0
        for (g0r, n, _) in GGROUPS:
            wfs = fw[g0r:g0r + n].reshape(-1)
            wbs = bw[g0r:g0r + n].reshape(-1)
            idx_parts.append(np.concatenate([wfs, wbs]) // 2)
            m[0:K, coff:coff + n * B] = (wfs & 1).astype(np.uint8)[None, :]
            m[K:128, coff:coff + n * B] = (wbs & 1).astype(np.uint8)[None, :]
            coff += n * B
        idx = _wrap16(np.concatenate(idx_parts).astype(np.int16))

        in_maps.append({
            "idx": np.ascontiguousarray(idx),
            "msk": np.ascontiguousarray(m),
            "g0i": np.ascontiguousarray(g0i),
            "bd": bd, "wrr": wrr, "p0": p0, "lnc": lnc,
            "ep2": Ep2,
        })
    return in_maps


def kernel(words, WA, ThetaB, E):
    nc = _get_nc()
    in_maps = _make_in_maps(words, WA, ThetaB, E)
    res = run_bass_kernel_spmd(nc, in_maps, list(range(N_CORES)))
    return np.concatenate(
        [res.results[c]["out"][0] for c in range(N_CORES)]).astype(np.float32)
